# revision 22
# baseline (speedup 1.0000x reference)
"""NodeGTransformer Trainium2 kernel.

Data-parallel across the batch: graph b -> NeuronCore b (B=8, 8 cores).

Per-core math (N=2048 nodes, U=256 features), with layouts chosen so no
on-device transposes are ever needed:
  NM (node-major)    tiles: [128 nodes, 256 feat]
  FM (feature-major) tiles: [128 feat, n free]

  h   = x @ W_lin                                   (FM lhsT, NM out)
  q/k/v chains: 6x { xn = SpMM (NM->FM); cur = silu(xn @ W) (FM->NM) }
  q,k chains end FM directly (lhsT/rhs roles swap on the last dense layer).
  pk  = silu(k @ Wk_att)                            (kT FM -> NM)
  kv0T = sum_n v[n,:]^T pk[n,:]                     (NM,NM -> [f,d])
  attn = q @ (Wq_att @ kv0 @ Wv_att @ Wo_att)       (small matmuls + FM out)
  out  = silu((attn + x) @ (bn_scale*W_proj) + b') + x @ W_res + b_res

Precision: activations grow ~8x per message-passing hop (up to ~1e6, and
~1e18 after the linear-attention contraction), so everything on device is
fp16 with per-tensor power-of-2 scale factors chosen from a host fp32
pre-pass (batch max -> one shared program for all cores, overflow-safe).
fp16 keeps 11 mantissa bits vs bf16's 8; adj is binary {0,1} so it is
exact in fp16, and all matmul products accumulate in fp32 PSUM.  The
dominant residual error source is weight rounding, so the hot dense-layer
weights are split W = hi + 2^-11*lo into two fp16 matmuls per output.
BN (inference) is folded into W_proj on host.
"""

import numpy as np

import concourse.bass as bass
import concourse.bacc as bacc
import concourse.mybir as mybir
import concourse.tile as tile
from concourse.bass_utils import run_bass_kernel_spmd

F16N = np.float16
F16 = mybir.dt.float16
F32 = mybir.dt.float32
AFT = mybir.ActivationFunctionType
ALU = mybir.AluOpType

B, N, U = 8, 2048, 256
NT = N // 128   # 16 node tiles
MC = N // 512   # 4 moving chunks of the node dim
DH = U // 128   # 2 feature-half chunks
BN_EPS = 1e-3
LO_SCALE = 2048.0        # 2^11: lo part of split weights is stored * 2^11
LO_DESCALE = 1.0 / LO_SCALE

SPLIT_W = ["w_lin", "w_q0", "w_q1", "w_q2", "w_k0", "w_k1", "w_k2",
           "w_v0", "w_v1", "w_v2", "w_p"]
SINGLE_W = ["w_ka", "w_qt", "w_vo", "w_r"]


def _sc(m):
    """Power-of-2 scale s with m*s <= 2^14 (half of fp16 max, headroom)."""
    m = float(max(m, 1e-30))
    e = 14 - int(np.ceil(np.log2(m)))
    e = max(-80, min(80, e))
    return float(2.0 ** e)


def _silu(z):
    zc = np.clip(z, -60, 60)
    return np.where(z > 0, z / (1 + np.exp(-zc)), z * np.exp(zc) / (1 + np.exp(zc)))


def _w2(w):
    """[256,256] fp32 -> [2,128,256] (K-dim chunked onto partitions)."""
    return np.ascontiguousarray(np.asarray(w, np.float32).reshape(DH, 128, U))


def _wsplit(w):
    """[256,256] fp32 -> [2(hi/lo), 2, 128, 256] fp16 with lo pre-scaled."""
    w = np.asarray(w, np.float32)
    hi = w.astype(F16N).astype(np.float32)
    lo = ((w - hi) * LO_SCALE).astype(F16N).astype(np.float32)
    return np.stack([_w2(hi), _w2(lo)]).astype(F16N)


def _build(nc, sc, *, mask_is_ones, zero_b_lin, zero_b_mp, zero_b_res,
           debug=False):
    """sc: dict of power-of-2 scale floats from the host pre-pass."""
    adjt_d = nc.dram_tensor("adjt", [MC, NT, 128, 512], F16, kind="ExternalInput")
    xt_d = nc.dram_tensor("xt", [DH, 128, N], F16, kind="ExternalInput")
    wd = {}
    for n in SPLIT_W:
        wd[n] = nc.dram_tensor(n, [2, DH, 128, U], F16, kind="ExternalInput")
    for n in SINGLE_W:
        wd[n] = nc.dram_tensor(n, [DH, 128, U], F16, kind="ExternalInput")
    bp_d = nc.dram_tensor("bp", [DH, 128, 1], F32, kind="ExternalInput")
    br_d = nc.dram_tensor("br", [DH, 128, 1], F32, kind="ExternalInput")
    if not zero_b_lin or not zero_b_mp:
        brow_d = nc.dram_tensor("brow", [10, 1, U], F32, kind="ExternalInput")
        bfm_d = nc.dram_tensor("bfm", [10, DH, 128, 1], F32, kind="ExternalInput")
    if not mask_is_ones:
        mnm_d = nc.dram_tensor("mnm", [NT, 128, 1], F32, kind="ExternalInput")
        mfm_d = nc.dram_tensor("mfm", [1, N], F16, kind="ExternalInput")
    outt_d = nc.dram_tensor("outt", [DH, 128, N], F32, kind="ExternalOutput")
    if debug:
        dbg_d = nc.dram_tensor("dbg", [8, 128, 512], F32,
                               kind="ExternalOutput")

    with tile.TileContext(nc) as tc:
        with (
            tc.tile_pool(name="sb", bufs=1) as sb,
            tc.tile_pool(name="ps", bufs=4, space="PSUM") as ps,
        ):
            # ---- resident SBUF tensors ---------------------------------
            xt = []
            for dh in range(DH):
                t = sb.tile([128, N], F16, name=f"xt{dh}", tag="xt", bufs=2)
                nc.sync.dma_start(out=t, in_=xt_d[dh])
                xt.append(t)
            w = {}
            W_BUFS = 2 * len(SPLIT_W) * DH + len(SINGLE_W) * DH

            def load_w(names):
                for n in names:
                    if n in SPLIT_W:
                        w[n] = [[None] * DH, [None] * DH]
                        for part in range(2):
                            for dh in range(DH):
                                t = sb.tile([128, U], F16,
                                            name=f"{n}_{part}_{dh}",
                                            tag="w", bufs=W_BUFS)
                                nc.sync.dma_start(out=t, in_=wd[n][part, dh])
                                w[n][part][dh] = t
                    else:
                        w[n] = [None] * DH
                        for dh in range(DH):
                            t = sb.tile([128, U], F16, name=f"{n}_{dh}",
                                        tag="w", bufs=W_BUFS)
                            nc.sync.dma_start(out=t, in_=wd[n][dh])
                            w[n][dh] = t

            # DMA order matters at startup: only w_lin is needed before the
            # first SpMM, and the first SpMM streams adjt[mc=0][*] — load
            # those first, the remaining weights and adj columns after.
            load_w(["w_lin", "w_q0"])
            adjt = [[None] * NT for _ in range(MC)]
            for mc in range(MC):
                for nt in range(NT):
                    t = sb.tile([128, 512], F16, name=f"adjt_{mc}_{nt}",
                                tag="adjt", bufs=MC * NT)
                    nc.sync.dma_start(out=t, in_=adjt_d[mc, nt])
                    adjt[mc][nt] = t
                if mc == 0:
                    load_w([n for n in SPLIT_W + SINGLE_W
                            if n not in ("w_lin", "w_q0")])
            bp_sb, br_sb = [], []
            for dh in range(DH):
                t = sb.tile([128, 1], F32, name=f"bp{dh}", tag="bias", bufs=4)
                nc.sync.dma_start(out=t, in_=bp_d[dh])
                bp_sb.append(t)
                t2 = sb.tile([128, 1], F32, name=f"br{dh}", tag="bias2", bufs=4)
                nc.sync.dma_start(out=t2, in_=br_d[dh])
                br_sb.append(t2)
            brow, bfm, ones_col = None, None, None
            if not zero_b_lin or not zero_b_mp:
                brow, bfm = [], []
                for i in range(10):
                    t = sb.tile([1, U], F32, name=f"brow{i}", tag="brow", bufs=10)
                    nc.sync.dma_start(out=t, in_=brow_d[i])
                    brow.append(t)
                    per = []
                    for dh in range(DH):
                        t2 = sb.tile([128, 1], F32, name=f"bfm{i}_{dh}",
                                     tag="bfm", bufs=20)
                        nc.sync.dma_start(out=t2, in_=bfm_d[i, dh])
                        per.append(t2)
                    bfm.append(per)
                ones_col = sb.tile([1, 128], F16, name="ones_col", tag="ones",
                                   bufs=1)
                nc.vector.memset(ones_col, 1.0)
            mnm, mfm = None, None
            if not mask_is_ones:
                mnm = []
                for nt in range(NT):
                    t = sb.tile([128, 1], F32, name=f"mnm{nt}", tag="mnm",
                                bufs=NT)
                    nc.sync.dma_start(out=t, in_=mnm_d[nt])
                    mnm.append(t)
                mfm = sb.tile([128, N], F16, name="mfm", tag="mfm", bufs=1)
                src = mfm_d[0]
                bcast = bass.AP(tensor=src.tensor, offset=src.offset,
                                ap=[[0, 128]] + list(src.ap))
                nc.sync.dma_start(out=mfm, in_=bcast)

            def dbg_dump(idx, ap):
                if not debug:
                    return
                t = sb.tile([128, 512], F32, name=f"dbg{idx}", tag="dbg",
                            bufs=2)
                pshape = ap.shape
                nc.vector.tensor_copy(t[:pshape[0], :pshape[1]], ap)
                nc.sync.dma_start(out=dbg_d[idx, :pshape[0], :pshape[1]],
                                  in_=t[:pshape[0], :pshape[1]])

            def scaled_brow(i, scale):
                """bias row * psum-scale, as fp16 [1,U] for the bias matmul."""
                t = sb.tile([1, U], F16, name=f"brs{i}", tag="brs", bufs=4)
                nc.vector.tensor_scalar_mul(t, brow[i], float(scale))
                return t

            # ---- building blocks ---------------------------------------
            def dense_nm(xn, wpair, birow, psc, a_out, tag, bufs, silu):
                """FM input (scaled psc at psum) -> NM fp16 out scaled a_out.

                wpair: (hi, lo) weight tile lists.  out = act(z)*a_out where
                psum_hi + 2^-11*psum_lo = psc * z."""
                whi, wlo = wpair
                outs = []
                for nt in range(NT):
                    p_hi = ps.tile([128, U], F32, name=f"ph_{tag}_{nt}",
                                   tag="psd")
                    p_lo = ps.tile([128, U], F32, name=f"pl_{tag}_{nt}",
                                   tag="psd")
                    mc, off = nt // 4, (nt % 4) * 128
                    for dh in range(DH):
                        nc.tensor.matmul(
                            p_hi, xn[dh][mc][:, off:off + 128], whi[dh],
                            start=(dh == 0),
                            stop=(dh == DH - 1 and birow is None))
                    if birow is not None:
                        nc.tensor.matmul(p_hi, ones_col, birow,
                                         start=False, stop=True)
                    for dh in range(DH):
                        nc.tensor.matmul(
                            p_lo, xn[dh][mc][:, off:off + 128], wlo[dh],
                            start=(dh == 0), stop=(dh == DH - 1))
                    tlo = sb.tile([128, U], F32, name=f"tl_{tag}_{nt}",
                                  tag="tmpa", bufs=4)
                    nc.vector.tensor_scalar_mul(tlo, p_lo, LO_DESCALE)
                    tz = sb.tile([128, U], F32, name=f"tz_{tag}_{nt}",
                                 tag="tmpz", bufs=3)
                    nc.vector.tensor_add(tz, p_hi, tlo)
                    t = sb.tile([128, U], F16, name=f"{tag}_{nt}", tag=tag,
                                bufs=bufs)
                    if silu:
                        ta = sb.tile([128, U], F32, name=f"ta_{tag}_{nt}",
                                     tag="tmpa", bufs=4)
                        nc.scalar.activation(ta, tz, AFT.Silu,
                                             scale=float(1.0 / psc))
                        nc.vector.tensor_scalar_mul(t, ta, float(a_out))
                    else:
                        nc.vector.tensor_scalar_mul(
                            t, tz, float(a_out / psc))
                    outs.append(t)
                return outs

            def spmm(cur, a_prev, b_t):
                """NM cur (scale a_prev) -> FM xn tiles (scale b_t)."""
                xn = [[None] * MC for _ in range(DH)]
                for mc in range(MC):
                    for dh in range(DH):
                        p = ps.tile([128, 512], F32, name=f"s_{mc}_{dh}",
                                    tag="pss")
                        for nt in range(NT):
                            nc.tensor.matmul(
                                p, cur[nt][:, dh * 128:(dh + 1) * 128],
                                adjt[mc][nt],
                                start=(nt == 0), stop=(nt == NT - 1))
                        t = sb.tile([128, 512], F16, name=f"xn_{dh}_{mc}",
                                    tag="xn", bufs=12)
                        nc.vector.tensor_scalar_mul(t, p, float(b_t / a_prev))
                        xn[dh][mc] = t
                return xn

            def dense_fm(xn, wpair, bias_fm, psc, a_out, cname):
                """FM in -> FM fp16 out (scale a_out): silu(z+b), z at psc."""
                whi, wlo = wpair
                big = [sb.tile([128, N], F16, name=f"{cname}T{eh}", tag="qkt",
                               bufs=4) for eh in range(DH)]
                for eh in range(DH):
                    for mc in range(MC):
                        p_hi = ps.tile([128, 512], F32, name=f"fh_{eh}_{mc}",
                                       tag="pss")
                        p_lo = ps.tile([128, 512], F32, name=f"fl_{eh}_{mc}",
                                       tag="pss")
                        for dh in range(DH):
                            nc.tensor.matmul(
                                p_hi, whi[dh][:, eh * 128:(eh + 1) * 128],
                                xn[dh][mc],
                                start=(dh == 0), stop=(dh == DH - 1))
                        for dh in range(DH):
                            nc.tensor.matmul(
                                p_lo, wlo[dh][:, eh * 128:(eh + 1) * 128],
                                xn[dh][mc],
                                start=(dh == 0), stop=(dh == DH - 1))
                        tlo = sb.tile([128, 512], F32, name=f"tlf_{eh}_{mc}",
                                      tag="tmpf2", bufs=4)
                        nc.vector.tensor_scalar_mul(tlo, p_lo, LO_DESCALE)
                        tz = sb.tile([128, 512], F32, name=f"tzf_{eh}_{mc}",
                                     tag="tmpf", bufs=3)
                        nc.vector.tensor_add(tz, p_hi, tlo)
                        ta = sb.tile([128, 512], F32, name=f"taf_{eh}_{mc}",
                                     tag="tmpf2", bufs=4)
                        if bias_fm is None:
                            nc.scalar.activation(ta, tz, AFT.Silu,
                                                 scale=float(1.0 / psc))
                        else:
                            nc.scalar.activation(ta, tz, AFT.Silu,
                                                 scale=float(1.0 / psc),
                                                 bias=bias_fm[eh])
                        dst = big[eh][:, mc * 512:(mc + 1) * 512]
                        nc.vector.tensor_scalar_mul(dst, ta, float(a_out))
                        if not mask_is_ones:
                            nc.vector.tensor_mul(
                                dst, dst, mfm[:, mc * 512:(mc + 1) * 512])
                return big

            def chain(cname, h_tiles, lw, rowi, fm_end):
                cur = h_tiles
                a_prev = sc["h"]
                for step in range(6):
                    layer = step // 2
                    b_t = sc[f"xn_{cname}{step}"]
                    a_out = sc[f"cur_{cname}{step}"]
                    xn = spmm(cur, a_prev, b_t)
                    last = step == 5
                    if last and fm_end:
                        bias_fm = None if zero_b_mp else bfm[rowi + layer]
                        return dense_fm(xn, lw[layer], bias_fm, b_t, a_out,
                                        cname)
                    birow = None if zero_b_mp else \
                        scaled_brow(rowi + layer, b_t)
                    tag = cname if last else "cur"
                    # 16 live + a few slots of pipeline slack: dense(t)'s
                    # writes only begin after spmm(t) finished reading the
                    # previous generation, so 2 full generations are not
                    # needed.
                    bufs = NT if last else 20
                    cur = dense_nm(xn, lw[layer], birow, b_t, a_out, tag,
                                   bufs, silu=True)
                    if cname == "q" and step == 0:
                        dbg_dump(1, xn[0][0])
                        dbg_dump(2, cur[0])
                    if last and not mask_is_ones:
                        for nt in range(NT):
                            nc.vector.tensor_scalar_mul(cur[nt], cur[nt],
                                                        mnm[nt])
                    a_prev = a_out
                return cur

            # ---- h = x @ W_lin + b_lin (no activation), scale a_h ------
            xt_fm = [[xt[dh][:, mc * 512:(mc + 1) * 512] for mc in range(MC)]
                     for dh in range(DH)]
            h = dense_nm(xt_fm, w["w_lin"],
                         None if zero_b_lin else scaled_brow(0, 1.0),
                         1.0, sc["h"], "h", NT, silu=False)
            dbg_dump(0, h[0])

            # ---- q/k/v message-passing chains --------------------------
            qT = chain("q", h, [w["w_q0"], w["w_q1"], w["w_q2"]], 1,
                       fm_end=True)
            kT = chain("k", h, [w["w_k0"], w["w_k1"], w["w_k2"]], 4,
                       fm_end=True)
            v = chain("v", h, [w["w_v0"], w["w_v1"], w["w_v2"]], 7,
                      fm_end=False)

            dbg_dump(3, qT[0][:, :512])
            dbg_dump(4, kT[0][:, :512])
            dbg_dump(5, v[0])

            a_q = sc["cur_q5"]
            a_k = sc["cur_k5"]
            a_v = sc["cur_v5"]

            # ---- pk = silu((k @ Wk_att) * mask), scale a_pk ------------
            a_pk = sc["pk"]
            if not mask_is_ones:
                mnm_k = []
                for nt in range(NT):
                    t = sb.tile([128, 1], F32, name=f"mnmk{nt}", tag="mnmk",
                                bufs=NT)
                    nc.vector.tensor_scalar_mul(t, mnm[nt], float(1.0 / a_k))
                    mnm_k.append(t)
            pk = []
            for nt in range(NT):
                p = ps.tile([128, U], F32, name=f"ppk_{nt}", tag="psd")
                for dh in range(DH):
                    nc.tensor.matmul(p, kT[dh][:, nt * 128:(nt + 1) * 128],
                                     w["w_ka"][dh],
                                     start=(dh == 0), stop=(dh == DH - 1))
                ta = sb.tile([128, U], F32, name=f"tapk_{nt}", tag="tmpa",
                             bufs=4)
                if mask_is_ones:
                    nc.scalar.activation(ta, p, AFT.Silu,
                                         scale=float(1.0 / a_k))
                else:
                    nc.scalar.activation(ta, p, AFT.Silu, scale=mnm_k[nt])
                t = sb.tile([128, U], F16, name=f"pk{nt}", tag="pk", bufs=NT)
                nc.vector.tensor_scalar_mul(t, ta, float(a_pk))
                pk.append(t)

            # vm = v * mask (the pv mask, hoisted past Wv_att)
            if mask_is_ones:
                vm = v
            else:
                vm = []
                for nt in range(NT):
                    t = sb.tile([128, U], F16, name=f"vm{nt}", tag="vm",
                                bufs=NT)
                    nc.vector.tensor_scalar_mul(t, v[nt], mnm[nt])
                    vm.append(t)

            # ---- kv0T[f,d] = sum_n vm[n,f] pk[n,d], psum scale a_v*a_pk
            a_kv = sc["kv0"]
            kv0T = []
            for fh in range(DH):
                p = ps.tile([128, U], F32, name=f"pkv0_{fh}", tag="psd")
                for nt in range(NT):
                    nc.tensor.matmul(p, vm[nt][:, fh * 128:(fh + 1) * 128],
                                     pk[nt],
                                     start=(nt == 0), stop=(nt == NT - 1))
                t = sb.tile([128, U], F16, name=f"kv0T{fh}", tag="small",
                            bufs=8)
                nc.vector.tensor_scalar_mul(t, p, float(a_kv / (a_v * a_pk)))
                kv0T.append(t)

            # ---- S1 = kv0 @ (Wv_att @ Wo_att);  KV2 = Wq_att @ S1 ------
            a_s1 = sc["S1"]
            S1 = []
            for dh in range(DH):
                p = ps.tile([128, U], F32, name=f"ps1_{dh}", tag="psd")
                for fh in range(DH):
                    nc.tensor.matmul(p, kv0T[fh][:, dh * 128:(dh + 1) * 128],
                                     w["w_vo"][fh],
                                     start=(fh == 0), stop=(fh == DH - 1))
                t = sb.tile([128, U], F16, name=f"S1_{dh}", tag="small",
                            bufs=8)
                nc.vector.tensor_scalar_mul(t, p, float(a_s1 / a_kv))
                S1.append(t)
            a_k2 = sc["KV2"]
            KV2 = []
            for ch in range(DH):
                p = ps.tile([128, U], F32, name=f"pkv2_{ch}", tag="psd")
                for dh in range(DH):
                    nc.tensor.matmul(
                        p, w["w_qt"][dh][:, ch * 128:(ch + 1) * 128], S1[dh],
                        start=(dh == 0), stop=(dh == DH - 1))
                t = sb.tile([128, U], F16, name=f"KV2_{ch}", tag="small",
                            bufs=8)
                nc.vector.tensor_scalar_mul(t, p, float(a_k2 / a_s1))
                KV2.append(t)

            dbg_dump(6, pk[0])
            dbg_dump(7, KV2[0])

            # ---- ytT = (attnT + xT)*a_y; attnT psum scale a_k2*a_q -----
            a_y = sc["y"]
            yt = [sb.tile([128, N], F16, name=f"yt{eh}", tag="yt", bufs=2)
                  for eh in range(DH)]
            for eh in range(DH):
                for mc in range(MC):
                    p = ps.tile([128, 512], F32, name=f"pat_{eh}_{mc}",
                                tag="pss")
                    for ch in range(DH):
                        nc.tensor.matmul(
                            p, KV2[ch][:, eh * 128:(eh + 1) * 128],
                            qT[ch][:, mc * 512:(mc + 1) * 512],
                            start=(ch == 0), stop=(ch == DH - 1))
                    sl = slice(mc * 512, (mc + 1) * 512)
                    t1 = sb.tile([128, 512], F32, name=f"ty_{eh}_{mc}",
                                 tag="tmpf", bufs=3)
                    nc.vector.tensor_scalar_mul(
                        t1, p, float(a_y / (a_k2 * a_q)))
                    nc.vector.scalar_tensor_tensor(
                        yt[eh][:, sl], xt[eh][:, sl], float(a_y), t1,
                        ALU.mult, ALU.add)

            # ---- out = silu(y@Wp + bp) + x@Wr (+ br) -------------------
            wp_hi, wp_lo = w["w_p"]
            for gh in range(DH):
                for mc in range(MC):
                    sl = slice(mc * 512, (mc + 1) * 512)
                    p_hi = ps.tile([128, 512], F32, name=f"pjh_{gh}_{mc}",
                                   tag="pss")
                    p_lo = ps.tile([128, 512], F32, name=f"pjl_{gh}_{mc}",
                                   tag="pss")
                    for eh in range(DH):
                        nc.tensor.matmul(
                            p_hi, wp_hi[eh][:, gh * 128:(gh + 1) * 128],
                            yt[eh][:, sl],
                            start=(eh == 0), stop=(eh == DH - 1))
                    for eh in range(DH):
                        nc.tensor.matmul(
                            p_lo, wp_lo[eh][:, gh * 128:(gh + 1) * 128],
                            yt[eh][:, sl],
                            start=(eh == 0), stop=(eh == DH - 1))
                    tlo = sb.tile([128, 512], F32, name=f"tlp_{gh}_{mc}",
                                  tag="tmpf2", bufs=4)
                    nc.vector.tensor_scalar_mul(tlo, p_lo, LO_DESCALE)
                    tz = sb.tile([128, 512], F32, name=f"tzp_{gh}_{mc}",
                                 tag="tmpf", bufs=3)
                    nc.vector.tensor_add(tz, p_hi, tlo)
                    tp = sb.tile([128, 512], F32, name=f"tp_{gh}_{mc}",
                                 tag="tp", bufs=2)
                    nc.scalar.activation(tp, tz, AFT.Silu,
                                         scale=float(1.0 / a_y),
                                         bias=bp_sb[gh])
                    p2 = ps.tile([128, 512], F32, name=f"prs_{gh}_{mc}",
                                 tag="pss")
                    for dh in range(DH):
                        nc.tensor.matmul(
                            p2, w["w_r"][dh][:, gh * 128:(gh + 1) * 128],
                            xt[dh][:, sl],
                            start=(dh == 0), stop=(dh == DH - 1))
                    if not zero_b_res:
                        nc.scalar.activation(tp, tp, AFT.Copy, bias=br_sb[gh])
                    ot = sb.tile([128, 512], F32, name=f"ot_{gh}_{mc}",
                                 tag="ot", bufs=2)
                    nc.vector.tensor_add(ot, p2, tp)
                    nc.sync.dma_start(out=outt_d[gh, :, sl], in_=ot)
    return nc


def _host_prepass(x, adj, W_lin, Wq_mp, Wk_mp, Wv_mp, Wk_att, Wv_att,
                  Wq_att, Wo_att):
    """fp32 forward to find batch-max magnitudes for every scaled store."""
    mx = {}

    def upd(k, a):
        m = float(np.abs(a).max())
        if m > mx.get(k, 0.0):
            mx[k] = m

    WvWo = Wv_att @ Wo_att
    for b in range(B):
        xb, ab = x[b], adj[b]
        h = xb @ W_lin
        upd("h", h)
        chains = {}
        for c, Ws in (("q", Wq_mp), ("k", Wk_mp), ("v", Wv_mp)):
            cur = h
            for t in range(6):
                xn = ab @ cur
                upd(f"xn_{c}{t}", xn)
                cur = _silu(xn @ Ws[t // 2])
                upd(f"cur_{c}{t}", cur)
            chains[c] = cur
        pk = _silu(chains["k"] @ Wk_att)
        upd("pk", pk)
        kv0 = pk.T @ chains["v"]
        upd("kv0", kv0)
        S1 = kv0 @ WvWo
        upd("S1", S1)
        KV2 = Wq_att @ S1
        upd("KV2", KV2)
        attn = chains["q"] @ KV2
        upd("y", attn + xb)
    return {k: _sc(m) for k, m in mx.items()}


def prepare(x, adj, nodal_mask, W_lin, b_lin, Wq_mp, bq_mp, Wk_mp, bk_mp,
            Wv_mp, bv_mp, Wk_att, Wv_att, Wq_att, Wo_att,
            bn_gamma, bn_beta, bn_mean, bn_var, W_proj, b_proj, W_res, b_res):
    """Build the Bass program + per-core input maps. Returns
    (nc, in_maps, mask_is_ones, nodal_mask_f32)."""
    x = np.asarray(x, np.float32)
    adj = np.asarray(adj, np.float32)
    nodal_mask = np.asarray(nodal_mask, np.float32)

    mask_is_ones = bool(np.all(nodal_mask == 1.0))
    zero_b_lin = not np.any(b_lin)
    zero_b_mp = not (np.any(bq_mp) or np.any(bk_mp) or np.any(bv_mp))
    zero_b_res = not np.any(b_res)

    # host folds
    bn_scale = np.asarray(bn_gamma, np.float32) / np.sqrt(
        np.asarray(bn_var, np.float32) + BN_EPS)
    bn_shift = (np.asarray(bn_beta, np.float32)
                - np.asarray(bn_mean, np.float32) * bn_scale)
    W_proj = np.asarray(W_proj, np.float32)
    Wp = bn_scale[:, None] * W_proj
    bp = np.asarray(b_proj, np.float32) + bn_shift @ W_proj
    Wk_att = np.asarray(Wk_att, np.float32)
    Wv_att = np.asarray(Wv_att, np.float32)
    Wq_att = np.asarray(Wq_att, np.float32)
    Wo_att = np.asarray(Wo_att, np.float32)
    W_lin = np.asarray(W_lin, np.float32)
    Wq_mp = np.asarray(Wq_mp, np.float32)
    Wk_mp = np.asarray(Wk_mp, np.float32)
    Wv_mp = np.asarray(Wv_mp, np.float32)
    WvWo = Wv_att @ Wo_att
    WqT = np.ascontiguousarray(Wq_att.T)

    # If the mask is not all-ones it zeroes chain outputs, which only
    # shrinks maxima, so the all-ones pre-pass scales stay safe.
    scales = _host_prepass(x, adj, W_lin, Wq_mp, Wk_mp, Wv_mp,
                           Wk_att, Wv_att, Wq_att, Wo_att)

    wmats = {"w_lin": _wsplit(W_lin), "w_p": _wsplit(Wp)}
    for i in range(3):
        wmats[f"w_q{i}"] = _wsplit(Wq_mp[i])
        wmats[f"w_k{i}"] = _wsplit(Wk_mp[i])
        wmats[f"w_v{i}"] = _wsplit(Wv_mp[i])
    for nm, mat in (("w_ka", Wk_att), ("w_qt", WqT), ("w_vo", WvWo),
                    ("w_r", np.asarray(W_res, np.float32))):
        wmats[nm] = _w2(mat).astype(F16N)

    bp2 = np.ascontiguousarray(bp.reshape(DH, 128, 1)).astype(np.float32)
    br2 = np.ascontiguousarray(
        np.asarray(b_res, np.float32).reshape(DH, 128, 1))

    nc = bacc.Bacc()
    _build(nc, scales, mask_is_ones=mask_is_ones, zero_b_lin=zero_b_lin,
           zero_b_mp=zero_b_mp, zero_b_res=zero_b_res)
    nc.compile()

    shared = {"bp": bp2, "br": br2}
    shared.update(wmats)
    if not zero_b_lin or not zero_b_mp:
        rows = np.zeros((10, U), np.float32)
        rows[0] = np.asarray(b_lin, np.float32)
        for i in range(3):
            rows[1 + i] = np.asarray(bq_mp[i], np.float32)
            rows[4 + i] = np.asarray(bk_mp[i], np.float32)
            rows[7 + i] = np.asarray(bv_mp[i], np.float32)
        shared["brow"] = rows.reshape(10, 1, U)
        shared["bfm"] = np.ascontiguousarray(
            rows.reshape(10, DH, 128, 1)).astype(np.float32)

    in_maps = []
    for b in range(B):
        # adjT packed: [mc, nt, p, f] = adj[b].T[nt*128+p, mc*512+f]
        adjt = np.ascontiguousarray(
            adj[b].T.reshape(NT, 128, MC, 512).transpose(2, 0, 1, 3)
        ).astype(F16N)
        xtb = np.ascontiguousarray(x[b].T.reshape(DH, 128, N)).astype(F16N)
        m = {"adjt": adjt, "xt": xtb}
        m.update(shared)
        if not mask_is_ones:
            m["mnm"] = np.ascontiguousarray(
                nodal_mask[b].reshape(NT, 128, 1)).astype(np.float32)
            m["mfm"] = nodal_mask[b].reshape(1, N).astype(F16N)
        in_maps.append(m)

    return nc, in_maps, mask_is_ones, nodal_mask


def postprocess(results, mask_is_ones, nodal_mask):
    """results: per-core dicts with 'outt' [DH,128,N] -> [B,N,U] fp32."""
    out = np.empty((B, N, U), np.float32)
    for b in range(B):
        ot = np.asarray(results[b]["outt"], np.float32)
        out[b] = ot.reshape(U, N).T
    if not mask_is_ones:
        out *= nodal_mask[:, :, None]
    return out


def kernel(**inputs):
    nc, in_maps, mask_is_ones, nodal_mask = prepare(**inputs)
    res = run_bass_kernel_spmd(nc, in_maps, core_ids=list(range(B)))
    return postprocess(res.results, mask_is_ones, nodal_mask)


# revision 26
# speedup vs baseline: 1.0523x; 1.0523x over previous
"""NodeGTransformer Trainium2 kernel.

Data-parallel across the batch: graph b -> NeuronCore b (B=8, 8 cores).

Per-core math (N=2048 nodes, U=256 features), with layouts chosen so no
on-device transposes are ever needed:
  NM (node-major)    tiles: [128 nodes, 256 feat]
  FM (feature-major) tiles: [128 feat, n free]

  h   = x @ W_lin                                   (FM lhsT, NM out)
  q/k/v chains: 6x { xn = SpMM (NM->FM); cur = silu(xn @ W) (FM->NM) }
  q,k chains end FM directly (lhsT/rhs roles swap on the last dense layer).
  pk  = silu(k @ Wk_att)                            (kT FM -> NM)
  kv0T = sum_n v[n,:]^T pk[n,:]                     (NM,NM -> [f,d])
  attn = q @ (Wq_att @ kv0 @ Wv_att @ Wo_att)       (small matmuls + FM out)
  out  = silu((attn + x) @ (bn_scale*W_proj) + b') + x @ W_res + b_res

Precision: activations grow ~8x per message-passing hop (up to ~1e6, and
~1e18 after the linear-attention contraction), so everything on device is
fp16 with per-tensor power-of-2 scale factors chosen from a host fp32
pre-pass (batch max -> one shared program for all cores, overflow-safe).
fp16 keeps 11 mantissa bits vs bf16's 8; adj is binary {0,1} so it is
exact in fp16, and all matmul products accumulate in fp32 PSUM.  The
dominant residual error source is weight rounding, so the hot dense-layer
weights are split W = hi + 2^-11*lo into two fp16 matmuls per output.
BN (inference) is folded into W_proj on host.
"""

import numpy as np

import concourse.bass as bass
import concourse.bacc as bacc
import concourse.mybir as mybir
import concourse.tile as tile
from concourse.bass_utils import run_bass_kernel_spmd

F16N = np.float16
F16 = mybir.dt.float16
F32 = mybir.dt.float32
AFT = mybir.ActivationFunctionType
ALU = mybir.AluOpType

B, N, U = 8, 2048, 256
NT = N // 128   # 16 node tiles
MC = N // 512   # 4 moving chunks of the node dim
DH = U // 128   # 2 feature-half chunks
BN_EPS = 1e-3
LO_SCALE = 2048.0        # 2^11: lo part of split weights is stored * 2^11
LO_DESCALE = 1.0 / LO_SCALE

SPLIT_W = ["w_lin", "w_q0", "w_q1", "w_q2", "w_k0", "w_k1", "w_k2",
           "w_v0", "w_v1", "w_v2", "w_p"]
SINGLE_W = ["w_ka", "w_qt", "w_vo", "w_r"]


def _sc(m):
    """Power-of-2 scale s with m*s <= 2^14 (half of fp16 max, headroom)."""
    m = float(max(m, 1e-30))
    e = 14 - int(np.ceil(np.log2(m)))
    e = max(-80, min(80, e))
    return float(2.0 ** e)


def _silu(z):
    zc = np.clip(z, -60, 60)
    return np.where(z > 0, z / (1 + np.exp(-zc)), z * np.exp(zc) / (1 + np.exp(zc)))


def _w2(w):
    """[256,256] fp32 -> [2,128,256] (K-dim chunked onto partitions)."""
    return np.ascontiguousarray(np.asarray(w, np.float32).reshape(DH, 128, U))


def _wsplit(w):
    """[256,256] fp32 -> [2(hi/lo), 2, 128, 256] fp16 with lo pre-scaled."""
    w = np.asarray(w, np.float32)
    hi = w.astype(F16N).astype(np.float32)
    lo = ((w - hi) * LO_SCALE).astype(F16N).astype(np.float32)
    return np.stack([_w2(hi), _w2(lo)]).astype(F16N)


def _build(nc, sc, *, mask_is_ones, zero_b_lin, zero_b_mp, zero_b_res,
           debug=False):
    """sc: dict of power-of-2 scale floats from the host pre-pass."""
    adjt_d = nc.dram_tensor("adjt", [MC, NT, 128, 512], F16, kind="ExternalInput")
    xt_d = nc.dram_tensor("xt", [DH, 128, N], F16, kind="ExternalInput")
    wd = {}
    for n in SPLIT_W:
        wd[n] = nc.dram_tensor(n, [2, DH, 128, U], F16, kind="ExternalInput")
    for n in SINGLE_W:
        wd[n] = nc.dram_tensor(n, [DH, 128, U], F16, kind="ExternalInput")
    bp_d = nc.dram_tensor("bp", [DH, 128, 1], F32, kind="ExternalInput")
    br_d = nc.dram_tensor("br", [DH, 128, 1], F32, kind="ExternalInput")
    if not zero_b_lin or not zero_b_mp:
        brow_d = nc.dram_tensor("brow", [10, 1, U], F32, kind="ExternalInput")
        bfm_d = nc.dram_tensor("bfm", [10, DH, 128, 1], F32, kind="ExternalInput")
    if not mask_is_ones:
        mnm_d = nc.dram_tensor("mnm", [NT, 128, 1], F32, kind="ExternalInput")
        mfm_d = nc.dram_tensor("mfm", [1, N], F16, kind="ExternalInput")
    outt_d = nc.dram_tensor("outt", [DH, 128, N], F32, kind="ExternalOutput")
    if debug:
        dbg_d = nc.dram_tensor("dbg", [8, 128, 512], F32,
                               kind="ExternalOutput")

    with tile.TileContext(nc) as tc:
        with (
            tc.tile_pool(name="sb", bufs=1) as sb,
            tc.tile_pool(name="ps", bufs=4, space="PSUM") as ps,
        ):
            # ---- resident SBUF tensors ---------------------------------
            xt = []
            for dh in range(DH):
                t = sb.tile([128, N], F16, name=f"xt{dh}", tag="xt", bufs=2)
                nc.sync.dma_start(out=t, in_=xt_d[dh])
                xt.append(t)
            w = {}
            W_BUFS = 2 * len(SPLIT_W) * DH + len(SINGLE_W) * DH

            def load_w(names):
                for n in names:
                    if n in SPLIT_W:
                        w[n] = [[None] * DH, [None] * DH]
                        for part in range(2):
                            for dh in range(DH):
                                t = sb.tile([128, U], F16,
                                            name=f"{n}_{part}_{dh}",
                                            tag="w", bufs=W_BUFS)
                                nc.sync.dma_start(out=t, in_=wd[n][part, dh])
                                w[n][part][dh] = t
                    else:
                        w[n] = [None] * DH
                        for dh in range(DH):
                            t = sb.tile([128, U], F16, name=f"{n}_{dh}",
                                        tag="w", bufs=W_BUFS)
                            nc.sync.dma_start(out=t, in_=wd[n][dh])
                            w[n][dh] = t

            # DMA order matters at startup: only w_lin is needed before the
            # first SpMM, and the first SpMM streams adjt[mc=0][*] — load
            # those first, the remaining weights and adj columns after.
            load_w(["w_lin", "w_q0"])
            adjt = [[None] * NT for _ in range(MC)]
            for mc in range(MC):
                for nt in range(NT):
                    t = sb.tile([128, 512], F16, name=f"adjt_{mc}_{nt}",
                                tag="adjt", bufs=MC * NT)
                    nc.sync.dma_start(out=t, in_=adjt_d[mc, nt])
                    adjt[mc][nt] = t
                if mc == 0:
                    load_w([n for n in SPLIT_W + SINGLE_W
                            if n not in ("w_lin", "w_q0")])
            bp_sb, br_sb = [], []
            for dh in range(DH):
                t = sb.tile([128, 1], F32, name=f"bp{dh}", tag="bias", bufs=4)
                nc.sync.dma_start(out=t, in_=bp_d[dh])
                bp_sb.append(t)
                t2 = sb.tile([128, 1], F32, name=f"br{dh}", tag="bias2", bufs=4)
                nc.sync.dma_start(out=t2, in_=br_d[dh])
                br_sb.append(t2)
            brow, bfm, ones_col = None, None, None
            if not zero_b_lin or not zero_b_mp:
                brow, bfm = [], []
                for i in range(10):
                    t = sb.tile([1, U], F32, name=f"brow{i}", tag="brow", bufs=10)
                    nc.sync.dma_start(out=t, in_=brow_d[i])
                    brow.append(t)
                    per = []
                    for dh in range(DH):
                        t2 = sb.tile([128, 1], F32, name=f"bfm{i}_{dh}",
                                     tag="bfm", bufs=20)
                        nc.sync.dma_start(out=t2, in_=bfm_d[i, dh])
                        per.append(t2)
                    bfm.append(per)
                ones_col = sb.tile([1, 128], F16, name="ones_col", tag="ones",
                                   bufs=1)
                nc.vector.memset(ones_col, 1.0)
            mnm, mfm = None, None
            if not mask_is_ones:
                mnm = []
                for nt in range(NT):
                    t = sb.tile([128, 1], F32, name=f"mnm{nt}", tag="mnm",
                                bufs=NT)
                    nc.sync.dma_start(out=t, in_=mnm_d[nt])
                    mnm.append(t)
                mfm = sb.tile([128, N], F16, name="mfm", tag="mfm", bufs=1)
                src = mfm_d[0]
                bcast = bass.AP(tensor=src.tensor, offset=src.offset,
                                ap=[[0, 128]] + list(src.ap))
                nc.sync.dma_start(out=mfm, in_=bcast)

            def dbg_dump(idx, ap):
                if not debug:
                    return
                t = sb.tile([128, 512], F32, name=f"dbg{idx}", tag="dbg",
                            bufs=2)
                pshape = ap.shape
                nc.vector.tensor_copy(t[:pshape[0], :pshape[1]], ap)
                nc.sync.dma_start(out=dbg_d[idx, :pshape[0], :pshape[1]],
                                  in_=t[:pshape[0], :pshape[1]])

            def scaled_brow(i, scale):
                """bias row * psum-scale, as fp16 [1,U] for the bias matmul."""
                t = sb.tile([1, U], F16, name=f"brs{i}", tag="brs", bufs=4)
                nc.vector.tensor_scalar_mul(t, brow[i], float(scale))
                return t

            # ---- building blocks ---------------------------------------
            def dense_nm(xn, wpair, birow, psc, a_out, tag, bufs, silu):
                """FM input (scaled psc at psum) -> NM fp16 out scaled a_out.

                wpair: (hi, lo) weight tile lists.  xn = (full, small) where
                small = full * 2^-11, so hi and lo products accumulate in ONE
                psum group: psum = xn@hi + xn_small@lo_s = psc * z."""
                whi, wlo = wpair
                xf, xs = xn
                outs = []
                for nt in range(NT):
                    p = ps.tile([128, U], F32, name=f"ph_{tag}_{nt}",
                                tag="psd")
                    mc, off = nt // 4, (nt % 4) * 128
                    for dh in range(DH):
                        nc.tensor.matmul(
                            p, xf[dh][mc][:, off:off + 128], whi[dh],
                            start=(dh == 0), stop=False)
                    for dh in range(DH):
                        nc.tensor.matmul(
                            p, xs[dh][mc][:, off:off + 128], wlo[dh],
                            start=False,
                            stop=(dh == DH - 1 and birow is None))
                    if birow is not None:
                        nc.tensor.matmul(p, ones_col, birow,
                                         start=False, stop=True)
                    t = sb.tile([128, U], F16, name=f"{tag}_{nt}", tag=tag,
                                bufs=bufs)
                    if silu:
                        ta = sb.tile([128, U], F32, name=f"ta_{tag}_{nt}",
                                     tag="tmpa", bufs=4)
                        nc.scalar.activation(ta, p, AFT.Silu,
                                             scale=float(1.0 / psc))
                        nc.vector.tensor_scalar_mul(t, ta, float(a_out))
                    else:
                        nc.vector.tensor_scalar_mul(
                            t, p, float(a_out / psc))
                    outs.append(t)
                return outs

            def spmm(cur, a_prev, b_t):
                """NM cur (scale a_prev) -> FM (xn, xn_small) tile pairs."""
                xf = [[None] * MC for _ in range(DH)]
                xs = [[None] * MC for _ in range(DH)]
                for mc in range(MC):
                    for dh in range(DH):
                        p = ps.tile([128, 512], F32, name=f"s_{mc}_{dh}",
                                    tag="pss")
                        for nt in range(NT):
                            nc.tensor.matmul(
                                p, cur[nt][:, dh * 128:(dh + 1) * 128],
                                adjt[mc][nt],
                                start=(nt == 0), stop=(nt == NT - 1))
                        t = sb.tile([128, 512], F16, name=f"xn_{dh}_{mc}",
                                    tag="xn", bufs=8)
                        nc.vector.tensor_scalar_mul(t, p, float(b_t / a_prev))
                        ts2 = sb.tile([128, 512], F16, name=f"xs_{dh}_{mc}",
                                      tag="xs", bufs=8)
                        nc.scalar.activation(
                            ts2, p, AFT.Copy,
                            scale=float(b_t * LO_DESCALE / a_prev))
                        xf[dh][mc] = t
                        xs[dh][mc] = ts2
                return xf, xs

            def dense_fm(xn, wpair, bias_fm, psc, a_out, cname):
                """FM in -> FM fp16 out (scale a_out): silu(z+b), z at psc."""
                whi, wlo = wpair
                xf, xs = xn
                big = [sb.tile([128, N], F16, name=f"{cname}T{eh}", tag="qkt",
                               bufs=4) for eh in range(DH)]
                for eh in range(DH):
                    for mc in range(MC):
                        p = ps.tile([128, 512], F32, name=f"fh_{eh}_{mc}",
                                    tag="pss")
                        for dh in range(DH):
                            nc.tensor.matmul(
                                p, whi[dh][:, eh * 128:(eh + 1) * 128],
                                xf[dh][mc],
                                start=(dh == 0), stop=False)
                        for dh in range(DH):
                            nc.tensor.matmul(
                                p, wlo[dh][:, eh * 128:(eh + 1) * 128],
                                xs[dh][mc],
                                start=False, stop=(dh == DH - 1))
                        ta = sb.tile([128, 512], F32, name=f"taf_{eh}_{mc}",
                                     tag="tmpf2", bufs=4)
                        if bias_fm is None:
                            nc.scalar.activation(ta, p, AFT.Silu,
                                                 scale=float(1.0 / psc))
                        else:
                            nc.scalar.activation(ta, p, AFT.Silu,
                                                 scale=float(1.0 / psc),
                                                 bias=bias_fm[eh])
                        dst = big[eh][:, mc * 512:(mc + 1) * 512]
                        nc.vector.tensor_scalar_mul(dst, ta, float(a_out))
                        if not mask_is_ones:
                            nc.vector.tensor_mul(
                                dst, dst, mfm[:, mc * 512:(mc + 1) * 512])
                return big

            def chain(cname, h_tiles, lw, rowi, fm_end):
                cur = h_tiles
                a_prev = sc["h"]
                for step in range(6):
                    layer = step // 2
                    b_t = sc[f"xn_{cname}{step}"]
                    a_out = sc[f"cur_{cname}{step}"]
                    xn = spmm(cur, a_prev, b_t)
                    last = step == 5
                    if last and fm_end:
                        bias_fm = None if zero_b_mp else bfm[rowi + layer]
                        return dense_fm(xn, lw[layer], bias_fm, b_t, a_out,
                                        cname)
                    birow = None if zero_b_mp else \
                        scaled_brow(rowi + layer, b_t)
                    tag = cname if last else "cur"
                    # 16 live + a few slots of pipeline slack: dense(t)'s
                    # writes only begin after spmm(t) finished reading the
                    # previous generation, so 2 full generations are not
                    # needed.
                    bufs = NT if last else 20
                    cur = dense_nm(xn, lw[layer], birow, b_t, a_out, tag,
                                   bufs, silu=True)
                    if cname == "q" and step == 0:
                        dbg_dump(1, xn[0][0][0])
                        dbg_dump(2, cur[0])
                    if last and not mask_is_ones:
                        for nt in range(NT):
                            nc.vector.tensor_scalar_mul(cur[nt], cur[nt],
                                                        mnm[nt])
                    a_prev = a_out
                return cur

            # ---- h = x @ W_lin + b_lin (no activation), scale a_h ------
            xt_fm = [[xt[dh][:, mc * 512:(mc + 1) * 512] for mc in range(MC)]
                     for dh in range(DH)]
            xt_sm = [[None] * MC for _ in range(DH)]
            for dh in range(DH):
                for mc in range(MC):
                    t = sb.tile([128, 512], F16, name=f"xth_{dh}_{mc}",
                                tag="xs", bufs=8)
                    nc.scalar.activation(t, xt_fm[dh][mc], AFT.Copy,
                                         scale=LO_DESCALE)
                    xt_sm[dh][mc] = t
            h = dense_nm((xt_fm, xt_sm), w["w_lin"],
                         None if zero_b_lin else scaled_brow(0, 1.0),
                         1.0, sc["h"], "h", NT, silu=False)
            dbg_dump(0, h[0])

            # ---- q/k/v message-passing chains --------------------------
            qT = chain("q", h, [w["w_q0"], w["w_q1"], w["w_q2"]], 1,
                       fm_end=True)
            kT = chain("k", h, [w["w_k0"], w["w_k1"], w["w_k2"]], 4,
                       fm_end=True)
            v = chain("v", h, [w["w_v0"], w["w_v1"], w["w_v2"]], 7,
                      fm_end=False)

            dbg_dump(3, qT[0][:, :512])
            dbg_dump(4, kT[0][:, :512])
            dbg_dump(5, v[0])

            a_q = sc["cur_q5"]
            a_k = sc["cur_k5"]
            a_v = sc["cur_v5"]

            # ---- pk = silu((k @ Wk_att) * mask), scale a_pk ------------
            a_pk = sc["pk"]
            if not mask_is_ones:
                mnm_k = []
                for nt in range(NT):
                    t = sb.tile([128, 1], F32, name=f"mnmk{nt}", tag="mnmk",
                                bufs=NT)
                    nc.vector.tensor_scalar_mul(t, mnm[nt], float(1.0 / a_k))
                    mnm_k.append(t)
            pk = []
            for nt in range(NT):
                p = ps.tile([128, U], F32, name=f"ppk_{nt}", tag="psd")
                for dh in range(DH):
                    nc.tensor.matmul(p, kT[dh][:, nt * 128:(nt + 1) * 128],
                                     w["w_ka"][dh],
                                     start=(dh == 0), stop=(dh == DH - 1))
                ta = sb.tile([128, U], F32, name=f"tapk_{nt}", tag="tmpa",
                             bufs=4)
                if mask_is_ones:
                    nc.scalar.activation(ta, p, AFT.Silu,
                                         scale=float(1.0 / a_k))
                else:
                    nc.scalar.activation(ta, p, AFT.Silu, scale=mnm_k[nt])
                t = sb.tile([128, U], F16, name=f"pk{nt}", tag="pk", bufs=NT)
                nc.vector.tensor_scalar_mul(t, ta, float(a_pk))
                pk.append(t)

            # vm = v * mask (the pv mask, hoisted past Wv_att)
            if mask_is_ones:
                vm = v
            else:
                vm = []
                for nt in range(NT):
                    t = sb.tile([128, U], F16, name=f"vm{nt}", tag="vm",
                                bufs=NT)
                    nc.vector.tensor_scalar_mul(t, v[nt], mnm[nt])
                    vm.append(t)

            # ---- kv0T[f,d] = sum_n vm[n,f] pk[n,d], psum scale a_v*a_pk
            a_kv = sc["kv0"]
            kv0T = []
            for fh in range(DH):
                p = ps.tile([128, U], F32, name=f"pkv0_{fh}", tag="psd")
                for nt in range(NT):
                    nc.tensor.matmul(p, vm[nt][:, fh * 128:(fh + 1) * 128],
                                     pk[nt],
                                     start=(nt == 0), stop=(nt == NT - 1))
                t = sb.tile([128, U], F16, name=f"kv0T{fh}", tag="small",
                            bufs=8)
                nc.vector.tensor_scalar_mul(t, p, float(a_kv / (a_v * a_pk)))
                kv0T.append(t)

            # ---- S1 = kv0 @ (Wv_att @ Wo_att);  KV2 = Wq_att @ S1 ------
            a_s1 = sc["S1"]
            S1 = []
            for dh in range(DH):
                p = ps.tile([128, U], F32, name=f"ps1_{dh}", tag="psd")
                for fh in range(DH):
                    nc.tensor.matmul(p, kv0T[fh][:, dh * 128:(dh + 1) * 128],
                                     w["w_vo"][fh],
                                     start=(fh == 0), stop=(fh == DH - 1))
                t = sb.tile([128, U], F16, name=f"S1_{dh}", tag="small",
                            bufs=8)
                nc.vector.tensor_scalar_mul(t, p, float(a_s1 / a_kv))
                S1.append(t)
            a_k2 = sc["KV2"]
            KV2 = []
            for ch in range(DH):
                p = ps.tile([128, U], F32, name=f"pkv2_{ch}", tag="psd")
                for dh in range(DH):
                    nc.tensor.matmul(
                        p, w["w_qt"][dh][:, ch * 128:(ch + 1) * 128], S1[dh],
                        start=(dh == 0), stop=(dh == DH - 1))
                t = sb.tile([128, U], F16, name=f"KV2_{ch}", tag="small",
                            bufs=8)
                nc.vector.tensor_scalar_mul(t, p, float(a_k2 / a_s1))
                KV2.append(t)

            dbg_dump(6, pk[0])
            dbg_dump(7, KV2[0])

            # ---- ytT = (attnT + xT)*a_y; attnT psum scale a_k2*a_q -----
            a_y = sc["y"]
            yt = [sb.tile([128, N], F16, name=f"yt{eh}", tag="yt", bufs=2)
                  for eh in range(DH)]
            for eh in range(DH):
                for mc in range(MC):
                    p = ps.tile([128, 512], F32, name=f"pat_{eh}_{mc}",
                                tag="pss")
                    for ch in range(DH):
                        nc.tensor.matmul(
                            p, KV2[ch][:, eh * 128:(eh + 1) * 128],
                            qT[ch][:, mc * 512:(mc + 1) * 512],
                            start=(ch == 0), stop=(ch == DH - 1))
                    sl = slice(mc * 512, (mc + 1) * 512)
                    t1 = sb.tile([128, 512], F32, name=f"ty_{eh}_{mc}",
                                 tag="tmpf", bufs=3)
                    nc.vector.tensor_scalar_mul(
                        t1, p, float(a_y / (a_k2 * a_q)))
                    nc.vector.scalar_tensor_tensor(
                        yt[eh][:, sl], xt[eh][:, sl], float(a_y), t1,
                        ALU.mult, ALU.add)

            # ---- out = silu(y@Wp + bp) + x@Wr (+ br) -------------------
            wp_hi, wp_lo = w["w_p"]
            yt_sm = [[None] * MC for _ in range(DH)]
            for eh in range(DH):
                for mc in range(MC):
                    t = sb.tile([128, 512], F16, name=f"yts_{eh}_{mc}",
                                tag="xs", bufs=8)
                    nc.scalar.activation(
                        t, yt[eh][:, mc * 512:(mc + 1) * 512], AFT.Copy,
                        scale=LO_DESCALE)
                    yt_sm[eh][mc] = t
            for gh in range(DH):
                for mc in range(MC):
                    sl = slice(mc * 512, (mc + 1) * 512)
                    p = ps.tile([128, 512], F32, name=f"pjh_{gh}_{mc}",
                                tag="pss")
                    for eh in range(DH):
                        nc.tensor.matmul(
                            p, wp_hi[eh][:, gh * 128:(gh + 1) * 128],
                            yt[eh][:, sl],
                            start=(eh == 0), stop=False)
                    for eh in range(DH):
                        nc.tensor.matmul(
                            p, wp_lo[eh][:, gh * 128:(gh + 1) * 128],
                            yt_sm[eh][mc],
                            start=False, stop=(eh == DH - 1))
                    tp = sb.tile([128, 512], F32, name=f"tp_{gh}_{mc}",
                                 tag="tp", bufs=2)
                    nc.scalar.activation(tp, p, AFT.Silu,
                                         scale=float(1.0 / a_y),
                                         bias=bp_sb[gh])
                    p2 = ps.tile([128, 512], F32, name=f"prs_{gh}_{mc}",
                                 tag="pss")
                    for dh in range(DH):
                        nc.tensor.matmul(
                            p2, w["w_r"][dh][:, gh * 128:(gh + 1) * 128],
                            xt[dh][:, sl],
                            start=(dh == 0), stop=(dh == DH - 1))
                    if not zero_b_res:
                        nc.scalar.activation(tp, tp, AFT.Copy, bias=br_sb[gh])
                    ot = sb.tile([128, 512], F32, name=f"ot_{gh}_{mc}",
                                 tag="ot", bufs=2)
                    nc.vector.tensor_add(ot, p2, tp)
                    nc.sync.dma_start(out=outt_d[gh, :, sl], in_=ot)
    return nc


def _host_prepass(x, adj, W_lin, Wq_mp, Wk_mp, Wv_mp, Wk_att, Wv_att,
                  Wq_att, Wo_att):
    """fp32 forward to find batch-max magnitudes for every scaled store."""
    mx = {}

    def upd(k, a):
        m = float(np.abs(a).max())
        if m > mx.get(k, 0.0):
            mx[k] = m

    WvWo = Wv_att @ Wo_att
    for b in range(B):
        xb, ab = x[b], adj[b]
        h = xb @ W_lin
        upd("h", h)
        chains = {}
        for c, Ws in (("q", Wq_mp), ("k", Wk_mp), ("v", Wv_mp)):
            cur = h
            for t in range(6):
                xn = ab @ cur
                upd(f"xn_{c}{t}", xn)
                cur = _silu(xn @ Ws[t // 2])
                upd(f"cur_{c}{t}", cur)
            chains[c] = cur
        pk = _silu(chains["k"] @ Wk_att)
        upd("pk", pk)
        kv0 = pk.T @ chains["v"]
        upd("kv0", kv0)
        S1 = kv0 @ WvWo
        upd("S1", S1)
        KV2 = Wq_att @ S1
        upd("KV2", KV2)
        attn = chains["q"] @ KV2
        upd("y", attn + xb)
    return {k: _sc(m) for k, m in mx.items()}


def prepare(x, adj, nodal_mask, W_lin, b_lin, Wq_mp, bq_mp, Wk_mp, bk_mp,
            Wv_mp, bv_mp, Wk_att, Wv_att, Wq_att, Wo_att,
            bn_gamma, bn_beta, bn_mean, bn_var, W_proj, b_proj, W_res, b_res):
    """Build the Bass program + per-core input maps. Returns
    (nc, in_maps, mask_is_ones, nodal_mask_f32)."""
    x = np.asarray(x, np.float32)
    adj = np.asarray(adj, np.float32)
    nodal_mask = np.asarray(nodal_mask, np.float32)

    mask_is_ones = bool(np.all(nodal_mask == 1.0))
    zero_b_lin = not np.any(b_lin)
    zero_b_mp = not (np.any(bq_mp) or np.any(bk_mp) or np.any(bv_mp))
    zero_b_res = not np.any(b_res)

    # host folds
    bn_scale = np.asarray(bn_gamma, np.float32) / np.sqrt(
        np.asarray(bn_var, np.float32) + BN_EPS)
    bn_shift = (np.asarray(bn_beta, np.float32)
                - np.asarray(bn_mean, np.float32) * bn_scale)
    W_proj = np.asarray(W_proj, np.float32)
    Wp = bn_scale[:, None] * W_proj
    bp = np.asarray(b_proj, np.float32) + bn_shift @ W_proj
    Wk_att = np.asarray(Wk_att, np.float32)
    Wv_att = np.asarray(Wv_att, np.float32)
    Wq_att = np.asarray(Wq_att, np.float32)
    Wo_att = np.asarray(Wo_att, np.float32)
    W_lin = np.asarray(W_lin, np.float32)
    Wq_mp = np.asarray(Wq_mp, np.float32)
    Wk_mp = np.asarray(Wk_mp, np.float32)
    Wv_mp = np.asarray(Wv_mp, np.float32)
    WvWo = Wv_att @ Wo_att
    WqT = np.ascontiguousarray(Wq_att.T)

    # If the mask is not all-ones it zeroes chain outputs, which only
    # shrinks maxima, so the all-ones pre-pass scales stay safe.
    scales = _host_prepass(x, adj, W_lin, Wq_mp, Wk_mp, Wv_mp,
                           Wk_att, Wv_att, Wq_att, Wo_att)

    wmats = {"w_lin": _wsplit(W_lin), "w_p": _wsplit(Wp)}
    for i in range(3):
        wmats[f"w_q{i}"] = _wsplit(Wq_mp[i])
        wmats[f"w_k{i}"] = _wsplit(Wk_mp[i])
        wmats[f"w_v{i}"] = _wsplit(Wv_mp[i])
    for nm, mat in (("w_ka", Wk_att), ("w_qt", WqT), ("w_vo", WvWo),
                    ("w_r", np.asarray(W_res, np.float32))):
        wmats[nm] = _w2(mat).astype(F16N)

    bp2 = np.ascontiguousarray(bp.reshape(DH, 128, 1)).astype(np.float32)
    br2 = np.ascontiguousarray(
        np.asarray(b_res, np.float32).reshape(DH, 128, 1))

    nc = bacc.Bacc()
    _build(nc, scales, mask_is_ones=mask_is_ones, zero_b_lin=zero_b_lin,
           zero_b_mp=zero_b_mp, zero_b_res=zero_b_res)
    nc.compile()

    shared = {"bp": bp2, "br": br2}
    shared.update(wmats)
    if not zero_b_lin or not zero_b_mp:
        rows = np.zeros((10, U), np.float32)
        rows[0] = np.asarray(b_lin, np.float32)
        for i in range(3):
            rows[1 + i] = np.asarray(bq_mp[i], np.float32)
            rows[4 + i] = np.asarray(bk_mp[i], np.float32)
            rows[7 + i] = np.asarray(bv_mp[i], np.float32)
        shared["brow"] = rows.reshape(10, 1, U)
        shared["bfm"] = np.ascontiguousarray(
            rows.reshape(10, DH, 128, 1)).astype(np.float32)

    in_maps = []
    for b in range(B):
        # adjT packed: [mc, nt, p, f] = adj[b].T[nt*128+p, mc*512+f]
        adjt = np.ascontiguousarray(
            adj[b].T.reshape(NT, 128, MC, 512).transpose(2, 0, 1, 3)
        ).astype(F16N)
        xtb = np.ascontiguousarray(x[b].T.reshape(DH, 128, N)).astype(F16N)
        m = {"adjt": adjt, "xt": xtb}
        m.update(shared)
        if not mask_is_ones:
            m["mnm"] = np.ascontiguousarray(
                nodal_mask[b].reshape(NT, 128, 1)).astype(np.float32)
            m["mfm"] = nodal_mask[b].reshape(1, N).astype(F16N)
        in_maps.append(m)

    return nc, in_maps, mask_is_ones, nodal_mask


def postprocess(results, mask_is_ones, nodal_mask):
    """results: per-core dicts with 'outt' [DH,128,N] -> [B,N,U] fp32."""
    out = np.empty((B, N, U), np.float32)
    for b in range(B):
        ot = np.asarray(results[b]["outt"], np.float32)
        out[b] = ot.reshape(U, N).T
    if not mask_is_ones:
        out *= nodal_mask[:, :, None]
    return out


def kernel(**inputs):
    nc, in_maps, mask_is_ones, nodal_mask = prepare(**inputs)
    res = run_bass_kernel_spmd(nc, in_maps, core_ids=list(range(B)))
    return postprocess(res.results, mask_is_ones, nodal_mask)
